# revision 6
# baseline (speedup 1.0000x reference)
# EpLSTMCell Trainium2 kernel: 8-way tensor-parallel over the gate dimension
# with a per-step AllGather of the hidden state.
#
# Math (per step t):
#   g  = x_t @ Wx.T + h_{t-1} @ Wh.T + (bx + bh)      # gates, order I,F,G,O,R
#   ft = sigmoid(gF); gt = tanh(gG); ot = sigmoid(gO); rt = sigmoid(gR)
#   it = 1 - ft  (tied -> the I-gate slice of Wx/Wh/b is mathematically unused)
#   c  = ft*c + (1-ft)*gt + rt*tanh(m_t) = ft*(c-gt) + (gt + rt*tanh(m_t))
#   h  = ot*tanh(c);  out[t] = h
#
# Core q handles Dh slice [slc*q, slc*(q+1)) of gates, grouped into PSUM
# halves {G,R} (finishes first; gt/rt/s/q/w precomputed while the second
# half streams) and {F,O} (finishes last; short chain ft->fs->c->th->h).
# h_f slices are AllGathered UNTRANSPOSED; the DRAM->SBUF gather DMA does
# the transpose (XBAR dma_start_transpose) straight into the next step's
# stationary layout - no PE transpose, no PSUM->SBUF copy on the chain.
#
# All weight reshapes/transposes/casts and tanh(memories) are host-side numpy.

import os

import numpy as np
import ml_dtypes

T, B, DX, DH = 128, 128, 1024, 1024
NC = 8
KT = DX // 128       # 8 contraction tiles
NG = 4               # gates kept: G, R, F, O (I dropped: it = 1-ft)
SLC = DH // NC       # 128: Dh slice per core
GW = NG * SLC        # 512 gate columns per core
HALF = GW // 2       # 256: {G,R} | {F,O}
# 5-gate chunk indices in Wx/Wh/b (order I,F,G,O,R) for our col order G,R,F,O
GATE_CHUNKS = (2, 4, 1, 3)

_NC_CACHE = {}


def _build_nc(n_steps=T, repeat=1, use_ag=True, nsub=1):
    import concourse.mybir as mybir
    import concourse.tile as tile
    from concourse import bacc

    bl = B // nsub           # batch rows per stream
    f32 = mybir.dt.float32
    bf16 = mybir.dt.bfloat16
    Sig = mybir.ActivationFunctionType.Sigmoid
    Tanh = mybir.ActivationFunctionType.Tanh

    nc = bacc.Bacc(
        "TRN2",
        target_bir_lowering=False,
        debug=False,
        enable_asserts=False,
        num_devices=NC,
    )

    xT = nc.dram_tensor("xT", [n_steps, 128, KT, B], bf16, kind="ExternalInput").ap()
    h0T = nc.dram_tensor("h0T", [128, KT, B], bf16, kind="ExternalInput").ap()
    wxT = nc.dram_tensor("wxT", [128, KT, GW], bf16, kind="ExternalInput").ap()
    whT = nc.dram_tensor("whT", [128, KT, GW], bf16, kind="ExternalInput").ap()
    biasr = nc.dram_tensor("biasr", [1, GW], bf16, kind="ExternalInput").ap()
    mt = nc.dram_tensor("mt", [n_steps, B, SLC], f32, kind="ExternalInput").ap()
    c0 = nc.dram_tensor("c0", [B, SLC], f32, kind="ExternalInput").ap()
    out = nc.dram_tensor("out", [n_steps, B, SLC], bf16, kind="ExternalOutput").ap()

    groups = [list(range(NC))]

    with tile.TileContext(nc) as tc:
        with (
            tc.tile_pool(name="const", bufs=1) as constp,
            tc.tile_pool(name="xs", bufs=3) as xsp,
            tc.tile_pool(name="hs", bufs=2) as hsp,
            tc.tile_pool(name="ms", bufs=3) as msp,
            tc.tile_pool(name="cell", bufs=2) as cellp,
            tc.tile_pool(name="act", bufs=2) as actp,
            tc.tile_pool(name="ps", bufs=2, space="PSUM") as psp,
            tc.tile_pool(name="drin", bufs=2, space="DRAM") as drinp,
            tc.tile_pool(name="drout", bufs=2, space="DRAM") as droutp,
        ):
            wx_sb = constp.tile([128, KT, GW], bf16)
            nc.sync.dma_start(wx_sb[:], wxT[:])
            wh_sb = constp.tile([128, KT, GW], bf16)
            nc.sync.dma_start(wh_sb[:], whT[:])
            bias_sb = constp.tile([1, GW], bf16)
            nc.sync.dma_start(bias_sb[:], biasr[:])
            ones_sb = constp.tile([1, B], bf16)
            nc.gpsimd.memset(ones_sb[:], 1.0)

            # per-stream recurrent state
            c_prev = [None] * nsub
            h_sb = [None] * nsub
            for s in range(nsub):
                c_prev[s] = cellp.tile([bl, SLC], f32, tag=f"c{s}", name=f"c_init{s}")
                nc.sync.dma_start(c_prev[s][:], c0[s * bl : (s + 1) * bl, :])
                h_sb[s] = hsp.tile([128, KT, bl], bf16, tag=f"h{s}", name=f"h_init{s}")
                nc.sync.dma_start(h_sb[s][:], h0T[:, :, s * bl : (s + 1) * bl])

            def xpart(t, s, x_sb):
                # open gate accumulation for (stream s, step t): bias + x-proj.
                r0 = s * bl
                g_a = psp.tile([bl, HALF], f32, tag=f"ga{s}", name=f"ga{s}_{t}")
                g_b = psp.tile([bl, HALF], f32, tag=f"gb{s}", name=f"gb{s}_{t}")
                for g_ps, base in ((g_a, 0), (g_b, HALF)):
                    nc.tensor.matmul(
                        g_ps[:], ones_sb[:, r0 : r0 + bl],
                        bias_sb[:, base : base + HALF],
                        start=True, stop=False,
                    )
                for k in range(KT):
                    for g_ps, base in ((g_a, 0), (g_b, HALF)):
                        nc.tensor.matmul(
                            g_ps[:], x_sb[:, k, r0 : r0 + bl],
                            wx_sb[:, k, base : base + HALF],
                            start=False, stop=False,
                        )
                return g_a, g_b

            x_cur = xsp.tile([128, KT, B], bf16, tag="x", name="x_0")
            nc.gpsimd.dma_start(x_cur[:], xT[0])
            m_cur = msp.tile([B, SLC], f32, tag="m", name="m_0")
            nc.gpsimd.dma_start(m_cur[:], mt[0])

            g_cur = [xpart(0, s, x_cur) for s in range(nsub)]

            total_steps = n_steps * repeat
            for tt in range(total_steps):
                t = tt % n_steps
                last = tt == total_steps - 1
                tn = (tt + 1) % n_steps

                # prefetch x/m for step t+1 (gpsimd queue, before any AG)
                if not last:
                    x_nxt = xsp.tile([128, KT, B], bf16, tag="x", name=f"x_{tt + 1}")
                    nc.gpsimd.dma_start(x_nxt[:], xT[tn])
                    m_nxt = msp.tile([B, SLC], f32, tag="m", name=f"m_{tt + 1}")
                    nc.gpsimd.dma_start(m_nxt[:], mt[tn])

                g_nxt = [None] * nsub
                for s in range(nsub):
                    r0 = s * bl
                    g_a, g_b = g_cur[s]
                    # recurrent part: half {G,R} fully first, then {F,O}
                    for g_ps, base in ((g_a, 0), (g_b, HALF)):
                        for k in range(KT):
                            nc.tensor.matmul(
                                g_ps[:], h_sb[s][:, k, :],
                                wh_sb[:, k, base : base + HALF],
                                start=False,
                                stop=(k == KT - 1),
                            )

                    # early half {G,R}: precompute everything not needing F,O
                    gt = actp.tile([bl, SLC], f32, tag=f"gt{s}", name=f"gt{s}_{t}")
                    nc.scalar.activation(gt[:], g_a[:, 0:SLC], Tanh)
                    rt = actp.tile([bl, SLC], f32, tag=f"rt{s}", name=f"rt{s}_{t}")
                    nc.scalar.activation(rt[:], g_a[:, SLC : 2 * SLC], Sig)
                    s_ = actp.tile([bl, SLC], f32, tag=f"s{s}", name=f"s{s}_{t}")
                    nc.vector.tensor_sub(s_[:], c_prev[s][:], gt[:])
                    q = actp.tile([bl, SLC], f32, tag=f"q{s}", name=f"q{s}_{t}")
                    nc.vector.tensor_mul(q[:], rt[:], m_cur[r0 : r0 + bl, :])
                    w = actp.tile([bl, SLC], f32, tag=f"w{s}", name=f"w{s}_{t}")
                    nc.vector.tensor_add(w[:], gt[:], q[:])

                    # late half {F,O}: short chain to h
                    ft = actp.tile([bl, SLC], f32, tag=f"ft{s}", name=f"ft{s}_{t}")
                    nc.scalar.activation(ft[:], g_b[:, 0:SLC], Sig)
                    ot = actp.tile([bl, SLC], f32, tag=f"ot{s}", name=f"ot{s}_{t}")
                    nc.scalar.activation(ot[:], g_b[:, SLC : 2 * SLC], Sig)
                    fs = actp.tile([bl, SLC], f32, tag=f"fs{s}", name=f"fs{s}_{t}")
                    nc.vector.tensor_mul(fs[:], ft[:], s_[:])
                    c_new = cellp.tile([bl, SLC], f32, tag=f"c{s}", name=f"c{s}_{t}")
                    nc.vector.tensor_add(c_new[:], fs[:], w[:])
                    th = actp.tile([bl, SLC], f32, tag=f"th{s}", name=f"th{s}_{t}")
                    nc.scalar.activation(th[:], c_new[:], Tanh)
                    h_f = actp.tile([bl, SLC], bf16, tag=f"hf{s}", name=f"hf{s}_{t}")
                    nc.vector.tensor_mul(h_f[:], ot[:], th[:])
                    c_prev[s] = c_new

                    # chain: bounce -> AllGather -> transposing gather DMA
                    bounce = drinp.tile([bl, SLC], bf16, tag=f"bo{s}",
                                        name=f"bo{s}_{t}")
                    nc.sync.dma_start(bounce[:], h_f[:])
                    nc.sync.dma_start(out[t, r0 : r0 + bl, :], h_f[:])
                    if not last:
                        if use_ag:
                            gath = droutp.tile(
                                [NC * bl, SLC], bf16, addr_space="Shared",
                                tag=f"gath{s}", name=f"gath{s}_{t}",
                            )
                            nc.gpsimd.collective_compute(
                                "AllGather",
                                mybir.AluOpType.bypass,
                                replica_groups=groups,
                                ins=[bounce.opt()],
                                outs=[gath.opt()],
                            )
                            gsrc = gath
                        else:
                            # timing variant: keep the bounce->gather dep,
                            # drop the collective (math wrong for k>0)
                            gsrc = droutp.tile(
                                [NC * bl, SLC], bf16, tag=f"gath{s}",
                                name=f"gath{s}_{t}",
                            )
                            nc.sync.dma_start(gsrc[0:bl, :], bounce[:])
                        h_new = hsp.tile([128, KT, bl], bf16, tag=f"h{s}",
                                         name=f"h{s}_{t}")
                        h_flat = h_new[:].rearrange("p k b -> p (k b)")
                        cut = 2 * bl
                        nc.sync.dma_start_transpose(
                            h_flat[:, 0:cut], gsrc[0:cut, :]
                        )
                        nc.sync.dma_start_transpose(
                            h_flat[:, cut : KT * bl], gsrc[cut : KT * bl, :]
                        )
                        h_sb[s] = h_new
                        g_nxt[s] = xpart(tn, s, x_nxt)
                if not last:
                    x_cur = x_nxt
                    m_cur = m_nxt
                    g_cur = g_nxt

    nc.compile()
    return nc


def _get_nc(n_steps=T, repeat=1, use_ag=True, nsub=1):
    key = (n_steps, repeat, use_ag, nsub)
    if key not in _NC_CACHE:
        _NC_CACHE[key] = _build_nc(n_steps, repeat, use_ag, nsub)
    return _NC_CACHE[key]


def _prep_in_maps(inputs, n_steps=T, nsub=1):
    bf16 = ml_dtypes.bfloat16
    x = np.asarray(inputs["inputs"], np.float32)[:n_steps]
    m = np.asarray(inputs["memories"], np.float32)[:n_steps]
    h0 = np.asarray(inputs["h0"], np.float32)
    c0 = np.asarray(inputs["c0"], np.float32)
    Wx = np.asarray(inputs["Wx"], np.float32)
    bx = np.asarray(inputs["bx"], np.float32)
    Wh = np.asarray(inputs["Wh"], np.float32)
    bh = np.asarray(inputs["bh"], np.float32)
    bias = bx + bh

    # xT[t, p, k, b] = x[t, b, 128k+p]
    xTf = np.ascontiguousarray(
        x.reshape(n_steps, B, KT, 128).transpose(0, 3, 2, 1)
    ).astype(bf16)
    h0Tf = np.ascontiguousarray(h0.reshape(B, KT, 128).transpose(2, 1, 0)).astype(bf16)

    wxTq, whTq, biasq = [], [], []
    for q in range(NC):
        rows = np.concatenate(
            [np.arange(gc * DH + q * SLC, gc * DH + (q + 1) * SLC)
             for gc in GATE_CHUNKS]
        )
        wxTq.append(
            np.ascontiguousarray(
                Wx[rows].T.reshape(KT, 128, GW).transpose(1, 0, 2)
            ).astype(bf16)
        )
        whTq.append(
            np.ascontiguousarray(
                Wh[rows].T.reshape(KT, 128, GW).transpose(1, 0, 2)
            ).astype(bf16)
        )
        biasq.append(np.ascontiguousarray(bias[rows].reshape(1, GW)).astype(bf16))

    in_maps = []
    for j in range(NC):
        mtj = np.ascontiguousarray(
            np.tanh(m[:, :, j * SLC : (j + 1) * SLC])
        ).astype(np.float32)
        c0j = np.ascontiguousarray(c0[:, j * SLC : (j + 1) * SLC]).astype(np.float32)
        in_maps.append(
            {
                "xT": xTf,
                "h0T": h0Tf,
                "wxT": wxTq[j],
                "whT": whTq[j],
                "biasr": biasq[j],
                "mt": mtj,
                "c0": c0j,
            }
        )
    return in_maps


def _assemble(results, nsub=1):
    n_steps = results[0]["out"].shape[0]
    full = np.zeros((n_steps, B, DH), np.float32)
    for j in range(NC):
        full[:, :, j * SLC : (j + 1) * SLC] = results[j]["out"].astype(np.float32)
    return full


NSUB_DEFAULT = 1


def _run(inputs, n_steps=T, trace=False, nsub=NSUB_DEFAULT):
    from concourse import bass_utils

    nc = _get_nc(n_steps, 1, True, nsub)
    in_maps = _prep_in_maps(inputs, n_steps, nsub)
    res = bass_utils.run_bass_kernel_spmd(
        nc, in_maps, core_ids=list(range(NC)), trace=trace
    )
    return _assemble(res.results, nsub), res


def kernel(**inputs) -> np.ndarray:
    full, _ = _run(inputs, T, trace=bool(os.environ.get("EPLSTM_TRACE")))
    return full


# revision 23
# speedup vs baseline: 723.4016x; 723.4016x over previous
# EpLSTMCell Trainium2 kernel: 8-way tensor-parallel over the gate dimension
# with a per-step AllGather of the hidden state.
#
# Math (per step t):
#   g  = x_t @ Wx.T + h_{t-1} @ Wh.T + (bx + bh)      # gates, order I,F,G,O,R
#   ft = sigmoid(gF); gt = tanh(gG); ot = sigmoid(gO); rt = sigmoid(gR)
#   it = 1 - ft  (tied -> the I-gate slice of Wx/Wh/b is mathematically unused)
#   c  = ft*c + (1-ft)*gt + rt*tanh(m_t) = ft*(c-gt) + (gt + rt*tanh(m_t))
#   h  = ot*tanh(c);  out[t] = h
#
# Core q handles Dh slice [slc*q, slc*(q+1)) of gates, grouped into PSUM
# halves {G,R} (finishes first; gt/rt/s/q/w precomputed while the second
# half streams) and {F,O} (finishes last; short chain ft->fs->c->th->h).
# h_f slices are AllGathered UNTRANSPOSED; the DRAM->SBUF gather DMA does
# the transpose (XBAR dma_start_transpose) straight into the next step's
# stationary layout - no PE transpose, no PSUM->SBUF copy on the chain.
#
# All weight reshapes/transposes/casts and tanh(memories) are host-side numpy.

import os

import numpy as np
import ml_dtypes

T, B, DX, DH = 128, 128, 1024, 1024
NC = 8
KT = DX // 128       # 8 contraction tiles
NG = 4               # gates kept: G, R, F, O (I dropped: it = 1-ft)
SLC = DH // NC       # 128: Dh slice per core
GW = NG * SLC        # 512 gate columns per core
HALF = GW // 2       # 256: {G,R} | {F,O}
# 5-gate chunk indices in Wx/Wh/b (order I,F,G,O,R) for our col order G,R,F,O
GATE_CHUNKS = (2, 4, 1, 3)

_NC_CACHE = {}

# transposing-gather chunk boundaries (k-tile units)
GATHER_CHUNKS = ((0, 2), (2, KT))
# issue a dummy AllGather after each real one to keep the collective
# pipeline hot (the serial-AG latency includes an idle->wake cost)
DUMMY_AG = False


def _build_nc(n_steps=T, repeat=1, use_ag=True, nsub=1,
              gchunks=None, dummy_ag=None):
    import concourse.mybir as mybir
    import concourse.tile as tile
    from concourse import bacc

    if gchunks is None:
        gchunks = GATHER_CHUNKS
    if dummy_ag is None:
        dummy_ag = DUMMY_AG
    bl = B // nsub           # batch rows per stream
    f32 = mybir.dt.float32
    bf16 = mybir.dt.bfloat16
    Sig = mybir.ActivationFunctionType.Sigmoid
    Tanh = mybir.ActivationFunctionType.Tanh

    nc = bacc.Bacc(
        "TRN2",
        target_bir_lowering=False,
        debug=False,
        enable_asserts=False,
        num_devices=NC,
    )

    xT = nc.dram_tensor("xT", [n_steps, 128, KT, B], bf16, kind="ExternalInput").ap()
    h0T = nc.dram_tensor("h0T", [128, KT, B], bf16, kind="ExternalInput").ap()
    wxT = nc.dram_tensor("wxT", [128, KT, GW], bf16, kind="ExternalInput").ap()
    whT = nc.dram_tensor("whT", [128, KT, GW], bf16, kind="ExternalInput").ap()
    biasr = nc.dram_tensor("biasr", [1, GW], bf16, kind="ExternalInput").ap()
    mt = nc.dram_tensor("mt", [n_steps, B, SLC], f32, kind="ExternalInput").ap()
    c0 = nc.dram_tensor("c0", [B, SLC], f32, kind="ExternalInput").ap()
    out = nc.dram_tensor("out", [n_steps, B, SLC], bf16, kind="ExternalOutput").ap()

    groups = [list(range(NC))]

    with tile.TileContext(nc) as tc:
        with (
            tc.tile_pool(name="const", bufs=1) as constp,
            tc.tile_pool(name="xs", bufs=3) as xsp,
            tc.tile_pool(name="hs", bufs=2) as hsp,
            tc.tile_pool(name="ms", bufs=3) as msp,
            tc.tile_pool(name="cell", bufs=2) as cellp,
            tc.tile_pool(name="act", bufs=2) as actp,
            tc.tile_pool(name="ps", bufs=2, space="PSUM") as psp,
            tc.tile_pool(name="drin", bufs=2, space="DRAM") as drinp,
            tc.tile_pool(name="drout", bufs=2, space="DRAM") as droutp,
            tc.tile_pool(name="drdum", bufs=1, space="DRAM") as drdump,
            tc.tile_pool(name="drdumo", bufs=2, space="DRAM") as drdumop,
        ):
            wx_sb = constp.tile([128, KT, GW], bf16)
            nc.sync.dma_start(wx_sb[:], wxT[:])
            wh_sb = constp.tile([128, KT, GW], bf16)
            nc.sync.dma_start(wh_sb[:], whT[:])
            bias_sb = constp.tile([1, GW], bf16)
            nc.sync.dma_start(bias_sb[:], biasr[:])
            ones_sb = constp.tile([1, B], bf16)
            nc.gpsimd.memset(ones_sb[:], 1.0)
            if dummy_ag:
                # content is irrelevant; one row initialized so the tile has
                # a writer for dependency tracking
                dum_in = drdump.tile([16, SLC], bf16, tag="dumin", name="dumin")
                nc.sync.dma_start(dum_in[0:1, :], biasr[:, 0:SLC])

            # per-stream recurrent state
            c_prev = [None] * nsub
            h_sb = [None] * nsub
            for s in range(nsub):
                c_prev[s] = cellp.tile([bl, SLC], f32, tag=f"c{s}", name=f"c_init{s}")
                nc.sync.dma_start(c_prev[s][:], c0[s * bl : (s + 1) * bl, :])
                h_sb[s] = hsp.tile([128, KT, bl], bf16, tag=f"h{s}", name=f"h_init{s}")
                nc.sync.dma_start(h_sb[s][:], h0T[:, :, s * bl : (s + 1) * bl])

            def xpart(t, s, x_sb):
                # open gate accumulation for (stream s, step t): bias + x-proj.
                r0 = s * bl
                g_a = psp.tile([bl, HALF], f32, tag=f"ga{s}", name=f"ga{s}_{t}")
                g_b = psp.tile([bl, HALF], f32, tag=f"gb{s}", name=f"gb{s}_{t}")
                for g_ps, base in ((g_a, 0), (g_b, HALF)):
                    nc.tensor.matmul(
                        g_ps[:], ones_sb[:, r0 : r0 + bl],
                        bias_sb[:, base : base + HALF],
                        start=True, stop=False,
                    )
                for k in range(KT):
                    for g_ps, base in ((g_a, 0), (g_b, HALF)):
                        nc.tensor.matmul(
                            g_ps[:], x_sb[:, k, r0 : r0 + bl],
                            wx_sb[:, k, base : base + HALF],
                            start=False, stop=False,
                        )
                return g_a, g_b

            def load_m(t, tt):
                tiles = []
                for s in range(nsub):
                    mt_s = msp.tile([bl, SLC], f32, tag=f"m{s}", name=f"m{s}_{tt}")
                    nc.gpsimd.dma_start(mt_s[:], mt[t, s * bl : (s + 1) * bl, :])
                    tiles.append(mt_s)
                return tiles

            x_cur = xsp.tile([128, KT, B], bf16, tag="x", name="x_0")
            nc.gpsimd.dma_start(x_cur[:], xT[0])
            m_cur = load_m(0, 0)

            g_cur = [xpart(0, s, x_cur) for s in range(nsub)]

            total_steps = n_steps * repeat
            for tt in range(total_steps):
                t = tt % n_steps
                last = tt == total_steps - 1
                tn = (tt + 1) % n_steps

                # prefetch x/m for step t+1 (gpsimd queue, before any AG)
                if not last:
                    x_nxt = xsp.tile([128, KT, B], bf16, tag="x", name=f"x_{tt + 1}")
                    nc.gpsimd.dma_start(x_nxt[:], xT[tn])
                    m_nxt = load_m(tn, tt + 1)

                g_nxt = [None] * nsub
                for s in range(nsub):
                    r0 = s * bl
                    g_a, g_b = g_cur[s]
                    # recurrent part: half {G,R} fully first, then {F,O}
                    for g_ps, base in ((g_a, 0), (g_b, HALF)):
                        for k in range(KT):
                            nc.tensor.matmul(
                                g_ps[:], h_sb[s][:, k, :],
                                wh_sb[:, k, base : base + HALF],
                                start=False,
                                stop=(k == KT - 1),
                            )

                    # All activations are Tanh (sigmoid(x) = 0.5*tanh(x/2)+0.5
                    # with the /2 pre-scales folded into host-side weights and
                    # the post-affine folded into fused DVE ops). Cell state is
                    # carried half-scale (C2 = c/2); h is carried double-scale
                    # (H = 2h, halved on the host).
                    Mult = mybir.AluOpType.mult
                    Add = mybir.AluOpType.add
                    # early half {G,R}: one fused tanh + everything not needing F,O
                    ta = actp.tile([bl, HALF], f32, tag=f"ta{s}", name=f"ta{s}_{t}")
                    nc.scalar.activation(ta[:], g_a[:], Tanh)
                    tG, tR = ta[:, 0:SLC], ta[:, SLC : 2 * SLC]
                    # s2 = C2_prev - 0.5*tG = 0.5*(c_prev - gt)
                    s2 = actp.tile([bl, SLC], f32, tag=f"s{s}", name=f"s{s}_{t}")
                    nc.vector.scalar_tensor_tensor(
                        s2[:], tG, -0.5, c_prev[s][:], Mult, Add)
                    # q2 = (tR+1)*m4 = 0.5*rt*tanh(m)   (m4 = 0.25*tanh(m), host)
                    q2 = actp.tile([bl, SLC], f32, tag=f"q{s}", name=f"q{s}_{t}")
                    nc.vector.scalar_tensor_tensor(
                        q2[:], tR, 1.0, m_cur[s][:], Add, Mult)
                    # w2 = 0.5*tG + q2 = 0.5*(gt + rt*tanh(m))
                    w2 = actp.tile([bl, SLC], f32, tag=f"w{s}", name=f"w{s}_{t}")
                    nc.vector.scalar_tensor_tensor(w2[:], tG, 0.5, q2[:], Mult, Add)

                    # late half {F,O}: short chain to h
                    tF = actp.tile([bl, SLC], f32, tag=f"ft{s}", name=f"ft{s}_{t}")
                    nc.scalar.activation(tF[:], g_b[:, 0:SLC], Tanh)
                    tO = actp.tile([bl, SLC], f32, tag=f"ot{s}", name=f"ot{s}_{t}")
                    nc.scalar.activation(tO[:], g_b[:, SLC : 2 * SLC], Tanh)
                    # fs = (tF+1)*s2 = ft*(c_prev - gt)
                    fs = actp.tile([bl, SLC], f32, tag=f"fs{s}", name=f"fs{s}_{t}")
                    nc.vector.scalar_tensor_tensor(fs[:], tF, 1.0, s2[:], Add, Mult)
                    # C2_new = 0.5*fs + w2 = 0.5*c_new
                    c_new = cellp.tile([bl, SLC], f32, tag=f"c{s}", name=f"c{s}_{t}")
                    nc.vector.scalar_tensor_tensor(c_new[:], fs[:], 0.5, w2[:], Mult, Add)
                    # th = tanh(2*C2) = tanh(c)
                    th = actp.tile([bl, SLC], f32, tag=f"th{s}", name=f"th{s}_{t}")
                    nc.scalar.activation(th[:], c_new[:], Tanh, scale=2.0)
                    # H = (tO+1)*th = 2*h  (host multiplies by 0.5)
                    h_f = actp.tile([bl, SLC], bf16, tag=f"hf{s}", name=f"hf{s}_{t}")
                    nc.vector.scalar_tensor_tensor(h_f[:], tO, 1.0, th[:], Add, Mult)
                    c_prev[s] = c_new

                    # chain: bounce -> AllGather -> transposing gather DMA
                    bounce = drinp.tile([bl, SLC], bf16, tag=f"bo{s}",
                                        name=f"bo{s}_{t}")
                    nc.sync.dma_start(bounce[:], h_f[:])
                    nc.sync.dma_start(out[t, r0 : r0 + bl, :], h_f[:])
                    if not last:
                        if use_ag:
                            gath = droutp.tile(
                                [NC * bl, SLC], bf16, addr_space="Shared",
                                tag=f"gath{s}", name=f"gath{s}_{t}",
                            )
                            nc.gpsimd.collective_compute(
                                "AllGather",
                                mybir.AluOpType.bypass,
                                replica_groups=groups,
                                ins=[bounce.opt()],
                                outs=[gath.opt()],
                            )
                            if dummy_ag:
                                dg = drdumop.tile(
                                    [NC * 16, SLC], bf16, addr_space="Shared",
                                    tag=f"dg{s}", name=f"dg{s}_{t}",
                                )
                                nc.gpsimd.collective_compute(
                                    "AllGather",
                                    mybir.AluOpType.bypass,
                                    replica_groups=groups,
                                    ins=[dum_in.opt()],
                                    outs=[dg.opt()],
                                )
                            gsrc = gath
                        else:
                            # timing variant: keep the bounce->gather dep,
                            # drop the collective (math wrong for k>0)
                            gsrc = droutp.tile(
                                [NC * bl, SLC], bf16, tag=f"gath{s}",
                                name=f"gath{s}_{t}",
                            )
                            nc.sync.dma_start(gsrc[0:bl, :], bounce[:])
                        h_new = hsp.tile([128, KT, bl], bf16, tag=f"h{s}",
                                         name=f"h{s}_{t}")
                        h_flat = h_new[:].rearrange("p k b -> p (k b)")
                        # concurrent transposing chunks; small first chunk so
                        # the next step's k=0 matmuls start earliest
                        for c0, c1 in gchunks:
                            nc.sync.dma_start_transpose(
                                h_flat[:, c0 * bl : c1 * bl],
                                gsrc[c0 * bl : c1 * bl, :],
                            )
                        h_sb[s] = h_new
                        g_nxt[s] = xpart(tn, s, x_nxt)
                if not last:
                    x_cur = x_nxt
                    m_cur = m_nxt
                    g_cur = g_nxt

    nc.compile()
    return nc


def _get_nc(n_steps=T, repeat=1, use_ag=True, nsub=1, gchunks=None,
            dummy_ag=None):
    key = (n_steps, repeat, use_ag, nsub, gchunks, dummy_ag)
    if key not in _NC_CACHE:
        _NC_CACHE[key] = _build_nc(n_steps, repeat, use_ag, nsub, gchunks,
                                   dummy_ag)
    return _NC_CACHE[key]


def _prep_in_maps(inputs, n_steps=T, nsub=1):
    bf16 = ml_dtypes.bfloat16
    x = np.asarray(inputs["inputs"], np.float32)[:n_steps]
    m = np.asarray(inputs["memories"], np.float32)[:n_steps]
    h0 = np.asarray(inputs["h0"], np.float32)
    c0 = np.asarray(inputs["c0"], np.float32)
    Wx = np.asarray(inputs["Wx"], np.float32)
    bx = np.asarray(inputs["bx"], np.float32)
    Wh = np.asarray(inputs["Wh"], np.float32)
    bh = np.asarray(inputs["bh"], np.float32)
    bias = bx + bh

    # xT[t, p, k, b] = x[t, b, 128k+p]
    xTf = np.ascontiguousarray(
        x.reshape(n_steps, B, KT, 128).transpose(0, 3, 2, 1)
    ).astype(bf16)
    # h is carried double-scale on device (H = 2h)
    h0Tf = np.ascontiguousarray(
        (2.0 * h0).reshape(B, KT, 128).transpose(2, 1, 0)
    ).astype(bf16)

    # all-tanh gate trick: sigmoid gates (R,F,O) get their pre-activation
    # halved (sigmoid(x) = 0.5*tanh(x/2)+0.5); the h input is H = 2h, so the
    # h-weights absorb another 0.5 on every gate.
    xscale = np.repeat([1.0, 0.5, 0.5, 0.5], SLC).astype(np.float32)  # G,R,F,O
    wxTq, whTq, biasq = [], [], []
    for q in range(NC):
        rows = np.concatenate(
            [np.arange(gc * DH + q * SLC, gc * DH + (q + 1) * SLC)
             for gc in GATE_CHUNKS]
        )
        wxTq.append(
            np.ascontiguousarray(
                (Wx[rows].T * xscale[None, :]).reshape(KT, 128, GW).transpose(1, 0, 2)
            ).astype(bf16)
        )
        whTq.append(
            np.ascontiguousarray(
                (Wh[rows].T * (0.5 * xscale)[None, :])
                .reshape(KT, 128, GW).transpose(1, 0, 2)
            ).astype(bf16)
        )
        biasq.append(
            np.ascontiguousarray((bias[rows] * xscale).reshape(1, GW)).astype(bf16)
        )

    in_maps = []
    for j in range(NC):
        # m4 = 0.25*tanh(m); cell state carried half-scale (C2 = c/2)
        mtj = np.ascontiguousarray(
            0.25 * np.tanh(m[:, :, j * SLC : (j + 1) * SLC])
        ).astype(np.float32)
        c0j = np.ascontiguousarray(
            0.5 * c0[:, j * SLC : (j + 1) * SLC]
        ).astype(np.float32)
        in_maps.append(
            {
                "xT": xTf,
                "h0T": h0Tf,
                "wxT": wxTq[j],
                "whT": whTq[j],
                "biasr": biasq[j],
                "mt": mtj,
                "c0": c0j,
            }
        )
    return in_maps


def _assemble(results, nsub=1):
    n_steps = results[0]["out"].shape[0]
    full = np.zeros((n_steps, B, DH), np.float32)
    for j in range(NC):
        # device stores H = 2h
        full[:, :, j * SLC : (j + 1) * SLC] = 0.5 * results[j]["out"].astype(np.float32)
    return full


NSUB_DEFAULT = 1


def _run(inputs, n_steps=T, trace=False, nsub=NSUB_DEFAULT):
    from concourse import bass_utils

    nc = _get_nc(n_steps, 1, True, nsub)
    in_maps = _prep_in_maps(inputs, n_steps, nsub)
    res = bass_utils.run_bass_kernel_spmd(
        nc, in_maps, core_ids=list(range(NC)), trace=trace
    )
    return _assemble(res.results, nsub), res


def kernel(**inputs) -> np.ndarray:
    full, _ = _run(inputs, T, trace=bool(os.environ.get("EPLSTM_TRACE")))
    return full
